# revision 14
# baseline (speedup 1.0000x reference)
"""RegionLoss (YOLO-style pose loss) on 8 Trainium2 NeuronCores.

Strategy: pure data parallel over the batch (16 images/core). The loss is
extremely sparse: the x/y terms and all metrics only touch the <=5 GT cells
per image, so each core does
  * a dense pass over just the conf channel (sigmoid -> sum sq, count>0.5),
  * one indirect-DMA gather of the 19 needed channel values at each GT cell
    (host passes a channels-last copy so each cell's channels are contiguous),
  * tiny per-GT vector math + PE column-reductions to 7 partial scalars.
Host sums the 8x7 partials (the psum step) and assembles the outputs.

All transcendentals use the natural_log+exp ACT table set (sigmoid via
exp+reciprocal, sqrt via exp(0.5*ln)) so only one table load occurs, early
and off the critical path. The per-slot "valid" cumprod is a PE matmul with
a host-built triangular prefix mask.
"""
import numpy as np
import concourse.bass as bass
import concourse.tile as tile
from concourse import bacc, mybir
from concourse.bass_utils import run_bass_kernel_spmd
from concourse.tile_rust import add_dep_helper

K = 9
NH = NW = 76
HW = NH * NW              # 5776
NBC = 16                  # images per core
MAXGT = 5
NG = NBC * MAXGT          # 80
C = 2 * K + 1 + 1         # 20 channels
NCH = 2 * K + 1           # 19 gathered channels (class ch unused)
IMG_STRIDE = C * HW       # 115520 elements per image (either layout)
P = 128
FREE = NBC * HW // P      # 722
NCORES = 8
B = NBC * NCORES          # 128
AX = (640.0 / 76.0) ** 2
AY = (480.0 / 76.0) ** 2
CC9 = float(1.0 / (np.exp(2.0) - 1.0 + 1e-5) / 9.0)
E2CC9 = float(np.exp(2.0) * CC9)   # ce = exp(-d/40)*E2CC9 - CC9

F32 = mybir.dt.float32
I32 = mybir.dt.int32

Act = None  # set in build


def build_nc():
    nc = bacc.Bacc(None)
    AF = mybir.ActivationFunctionType
    OP = mybir.AluOpType
    tgtb = nc.dram_tensor("tgtb", [NG, 2 * K + 1], F32, kind="ExternalInput")
    confb = nc.dram_tensor("confb", [P, FREE], F32, kind="ExternalInput")
    tri = nc.dram_tensor("tri", [P, P], F32, kind="ExternalInput")
    xoutt = nc.dram_tensor("xoutt", [NBC, NH, NW, C], F32, kind="ExternalInput")
    partials = nc.dram_tensor("partials", [5, 2], F32, kind="ExternalOutput")

    with tile.TileContext(nc) as tc:
        with tc.tile_pool(name="sb", bufs=1) as sb, \
             tc.tile_pool(name="ps", bufs=1, space="PSUM") as ps:

            # preload the one table set (natural_log_exp_and_others, id 6)
            # containing every ACT func we use: exp, ln, square
            ldset = mybir.InstLoadActFuncSet(
                name=nc.get_next_instruction_name(), act_func_set_id=6,
                ins=[], outs=[])
            nc.scalar.add_instruction(ldset)

            # ---------------- critical chain: targets -> idx -> gather ----------------
            tgt_t = sb.tile([NG, 2 * K + 1], F32)
            nc.sync.dma_start(out=tgt_t[:], in_=tgtb[:])
            conf_t = sb.tile([P, FREE], F32)
            nc.sync.dma_start(out=conf_t[:], in_=confb[:])

            # fused floor of (x0*76, y0*76): cols 0 and K of tgt_t via stride-K AP
            xy = sb.tile([NG, 2], F32)
            nc.vector.tensor_scalar_mul(out=xy[:], in0=tgt_t[:, 0:K + 1:K],
                                        scalar1=float(NW))
            xyi = sb.tile([NG, 2], I32)
            xyf = sb.tile([NG, 2], F32)
            fx = sb.tile([NG, 2], F32)
            nc.vector.tensor_copy(out=xyi[:], in_=xy[:])
            nc.vector.tensor_copy(out=xyf[:], in_=xyi[:])
            nc.vector.tensor_tensor(out=fx[:], in0=xyf[:], in1=xy[:], op=OP.is_gt)
            nc.vector.tensor_tensor(out=xyf[:], in0=xyf[:], in1=fx[:], op=OP.subtract)
            cxf, cyf = xyf[:, 0:1], xyf[:, 1:2]

            # idx = (cyf*76 + cxf)*20 + bofs  (channels-last element offsets; f32 exact)
            basef = sb.tile([NG, 1], F32)
            nc.vector.scalar_tensor_tensor(out=basef[:], in0=cyf, scalar=float(NW),
                                           in1=cxf, op0=OP.mult, op1=OP.add)
            idxf = sb.tile([NG, 1], F32)
            nc.vector.scalar_tensor_tensor(out=idxf[:], in0=basef[:], scalar=float(C),
                                           in1=tgt_t[:, 2 * K:2 * K + 1],
                                           op0=OP.mult, op1=OP.add)
            idx = sb.tile([NG, 1], I32)
            idx_inst = nc.vector.tensor_copy(out=idx[:], in_=idxf[:])

            def after_idx(inst):
                add_dep_helper(inst.ins, idx_inst.ins,
                               reason="defer off-critical DVE work past idx")
                return inst

            # full gx, gy (off critical path; used for tx/ty)
            gx = sb.tile([NG, K], F32)
            gy = sb.tile([NG, K], F32)
            after_idx(nc.vector.tensor_scalar_mul(out=gx[:], in0=tgt_t[:, 0:K],
                                                  scalar1=float(NW)))
            after_idx(nc.vector.tensor_scalar_mul(out=gy[:], in0=tgt_t[:, K:2 * K],
                                                  scalar1=float(NH)))

            # indirect gather: g_t[g, :] = xoutt.flat[idx[g] : idx[g]+19]
            # (HW semantics: one index per partition, contiguous run per index)
            g_t = sb.tile([NG, NCH], F32)
            xflat = xoutt[:].rearrange("b h w c -> b (h w c)")
            nc.gpsimd.indirect_dma_start(
                out=g_t[:], out_offset=None, in_=xflat,
                in_offset=bass.IndirectOffsetOnAxis(ap=idx[:], axis=1))

            # ---------------- post-gather per-GT math ----------------
            # channels-last layout: col 2k = x_k, col 2k+1 = y_k, col 18 = conf
            # sigmoid(v) = 1/(1+exp(-v)) on cols 0,1 (x0,y0) and 18 (conf)
            e01 = sb.tile([NG, 2], F32)
            nc.scalar.activation(out=e01[:], in_=g_t[:, 0:2], func=AF.Exp, scale=-1.0)
            nc.vector.tensor_scalar_add(out=e01[:], in0=e01[:], scalar1=1.0)
            nc.vector.reciprocal(out=g_t[:, 0:2], in_=e01[:])
            esc = sb.tile([NG, 1], F32)
            nc.scalar.activation(out=esc[:], in_=g_t[:, 2 * K:2 * K + 1], func=AF.Exp,
                                 scale=-1.0)
            nc.vector.tensor_scalar_add(out=esc[:], in0=esc[:], scalar1=1.0)
            sc = sb.tile([NG, 1], F32)
            nc.vector.reciprocal(out=sc[:], in_=esc[:])

            gvals = sb.tile([P, 5], F32)
            nc.vector.memset(gvals[:], 0.0)
            tx = sb.tile([NG, K], F32)
            ty = sb.tile([NG, K], F32)
            after_idx(nc.vector.tensor_scalar(out=tx[:], in0=gx[:], scalar1=cxf[:, 0:1],
                                              scalar2=None, op0=OP.subtract))
            after_idx(nc.vector.tensor_scalar(out=ty[:], in0=gy[:], scalar1=cyf[:, 0:1],
                                              scalar2=None, op0=OP.subtract))
            dx = sb.tile([NG, K], F32)
            dy = sb.tile([NG, K], F32)
            nc.vector.tensor_tensor(out=dx[:], in0=g_t[:, 0:2 * K:2], in1=tx[:],
                                    op=OP.subtract)
            nc.vector.tensor_tensor(out=dy[:], in0=g_t[:, 1:2 * K + 1:2], in1=ty[:],
                                    op=OP.subtract)
            dx2 = sb.tile([NG, K], F32)
            dy2 = sb.tile([NG, K], F32)
            nc.vector.scalar_tensor_tensor(
                out=dx2[:], in0=dx[:], scalar=1.0, in1=dx[:],
                op0=OP.mult, op1=OP.mult, accum_out=gvals[0:NG, 0:1])
            nc.vector.scalar_tensor_tensor(
                out=dy2[:], in0=dy[:], scalar=1.0, in1=dy[:],
                op0=OP.mult, op1=OP.mult, accum_out=gvals[0:NG, 1:2])

            # corner confidence: d = sqrt(AX*dx2 + AY*dy2) = exp(0.5*ln(s2))
            dy2b = sb.tile([NG, K], F32)
            nc.vector.tensor_scalar_mul(out=dy2b[:], in0=dy2[:], scalar1=AY)
            s2 = sb.tile([NG, K], F32)
            nc.vector.scalar_tensor_tensor(out=s2[:], in0=dx2[:], scalar=AX, in1=dy2b[:],
                                           op0=OP.mult, op1=OP.add)
            lns = sb.tile([NG, K], F32)
            nc.scalar.activation(out=lns[:], in_=s2[:], func=AF.Ln)
            dd = sb.tile([NG, K], F32)
            nc.scalar.activation(out=dd[:], in_=lns[:], func=AF.Exp, scale=0.5)
            er = sb.tile([NG, K], F32)
            er_inst = nc.scalar.activation(out=er[:], in_=dd[:], func=AF.Exp,
                                           scale=-1.0 / 40.0)
            # mask d < 80  <=>  s2 < 6400
            mm = sb.tile([NG, K], F32)
            nc.vector.tensor_scalar(out=mm[:], in0=s2[:], scalar1=6400.0, scalar2=None,
                                    op0=OP.is_lt)
            # c/9 = (exp(2-d/40)-1)*CC9 = er*E2CC9 - CC9
            ce = sb.tile([NG, K], F32)
            nc.vector.tensor_scalar(out=ce[:], in0=er[:], scalar1=E2CC9, scalar2=CC9,
                                    op0=OP.mult, op1=OP.subtract)
            junk_g = sb.tile([NG, K], F32)
            confgt = sb.tile([NG, 1], F32)
            nc.vector.scalar_tensor_tensor(
                out=junk_g[:], in0=ce[:], scalar=1.0, in1=mm[:],
                op0=OP.mult, op1=OP.mult, accum_out=confgt[:])
            nc.vector.tensor_scalar(out=gvals[0:NG, 4:5], in0=confgt[:], scalar1=0.7,
                                    scalar2=None, op0=OP.is_gt)

            # conf correction 1 - 2*sigma(conf_logit)
            nc.vector.tensor_scalar(out=gvals[0:NG, 2:3], in0=sc[:], scalar1=-2.0,
                                    scalar2=1.0, op0=OP.mult, op1=OP.add)
            nc.vector.memset(gvals[0:NG, 3:4], 1.0)

            # ---------------- valid weights via PE prefix-count ----------------
            tri_t = sb.tile([P, P], F32)
            nc.sync.dma_start(out=tri_t[:], in_=tri[:])
            iz = sb.tile([P, 1], F32)
            nc.vector.memset(iz[:], 0.0)
            after_idx(nc.vector.tensor_scalar(out=iz[0:NG, 0:1], in0=tgt_t[:, 0:1],
                                              scalar1=0.0, scalar2=None, op0=OP.is_equal))
            psum_v = ps.tile([P, 1], F32)
            nc.tensor.matmul(out=psum_v[:], lhsT=tri_t[:], rhs=iz[:], start=True, stop=True)
            # valid = (prefix-zero-count == 0) = relu(1 - cnt); runs on ACT
            # (which can read PSUM) so it never stalls the DVE pipeline
            valid_w = sb.tile([P, 1], F32)
            one_b = nc.const_aps.tensor(1.0, (P, 1))
            nc.scalar.activation(out=valid_w[:], in_=psum_v[:], func=AF.Relu,
                                 scale=-1.0, bias=one_b)

            # ---------------- dense conf branch ----------------
            # sigma(z)^2 = exp(-2*ln(1+exp(-z))) — a pure-ACT 3-pass chain
            # (the +1 rides the Ln bias input), so it fills the ACT idle time
            # before the gather lands and never touches the DVE.
            dvals = sb.tile([P, 2], F32)
            ez = sb.tile([P, FREE], F32)
            nc.scalar.activation(out=ez[:], in_=conf_t[:], func=AF.Exp, scale=-1.0)
            lnz = sb.tile([P, FREE], F32)
            one_bd = nc.const_aps.tensor(1.0, (P, 1))
            nc.scalar.activation(out=lnz[:], in_=ez[:], func=AF.Ln, bias=one_bd)
            junk_d = sb.tile([P, FREE], F32)
            nc.scalar.activation(out=junk_d[:], in_=lnz[:], func=AF.Exp,
                                 scale=-2.0, accum_out=dvals[:, 0:1])
            junk_c = sb.tile([P, FREE], F32)
            nc.vector.tensor_scalar(
                out=junk_c[:], in0=conf_t[:], scalar1=0.0, scalar2=None,
                op0=OP.is_gt, op1=OP.add, accum_out=dvals[:, 1:2])

            ones = sb.tile([P, 1], F32)
            nc.vector.memset(ones[:], 1.0)

            # ---------------- reductions + output ----------------
            psum_g = ps.tile([5, 1], F32)
            psum_d = ps.tile([2, 1], F32)
            nc.tensor.matmul(out=psum_g[:], lhsT=gvals[:], rhs=valid_w[:],
                             start=True, stop=True)
            nc.tensor.matmul(out=psum_d[:], lhsT=dvals[:], rhs=ones[:],
                             start=True, stop=True)
            res = sb.tile([5, 2], F32)
            nc.vector.memset(res[:], 0.0)
            nc.vector.tensor_copy(out=res[0:5, 0:1], in_=psum_g[:])
            nc.vector.tensor_copy(out=res[0:2, 1:2], in_=psum_d[:])
            nc.sync.dma_start(out=partials[:], in_=res[:])
    nc.compile()
    return nc


def host_shards(output, target):
    """Split full inputs into per-core input maps (layout only, no math)."""
    output = np.ascontiguousarray(np.asarray(output, dtype=np.float32))
    target = np.ascontiguousarray(np.asarray(target, dtype=np.float32))
    g = np.arange(NG)
    bofs = ((g // MAXGT) * IMG_STRIDE).astype(np.float32)
    gb, gt_ = g[:, None] // MAXGT, g[:, None] % MAXGT
    tri = ((gb == gb.T) & (gt_ <= gt_.T)).astype(np.float32)
    tri_full = np.zeros((P, P), np.float32)
    tri_full[:NG, :NG] = tri
    maps = []
    for i in range(NCORES):
        ob = output[i * NBC:(i + 1) * NBC]
        confb = np.ascontiguousarray(ob[:, 2 * K].reshape(P, FREE))
        xoutt = np.ascontiguousarray(ob.transpose(0, 2, 3, 1))
        tb = target[i * NBC:(i + 1) * NBC].reshape(NBC, MAXGT, 2 * K + 3)
        tgtb = np.concatenate(
            [tb[:, :, 1:2 * K + 1:2], tb[:, :, 2:2 * K + 2:2]], axis=2
        ).reshape(NG, 2 * K)
        tgtb = np.ascontiguousarray(np.concatenate([tgtb, bofs[:, None]], axis=1))
        maps.append({"tgtb": tgtb, "confb": confb, "tri": tri_full, "xoutt": xoutt})
    return maps


def combine(partials_list):
    ps_ = np.stack([np.asarray(q).reshape(5, 2) for q in partials_list])
    p = ps_.sum(axis=0, dtype=np.float64).astype(np.float32)
    loss_x, loss_y, corr, ngt_cnt, ncorr_cnt = [np.float32(v) for v in p[:, 0]]
    sqsum, prop_cnt = np.float32(p[0, 1]), np.float32(p[1, 1])
    loss_conf = np.float32(sqsum + corr)
    loss = np.float32(np.float32(loss_x + loss_y) + loss_conf)
    nB = np.float32(B)
    return (loss, np.float32(ngt_cnt / nB), np.float32(ncorr_cnt / nB),
            np.float32(prop_cnt / nB), loss_x, loss_y, loss_conf)


_NC_CACHE = None


def _get_nc():
    global _NC_CACHE
    if _NC_CACHE is None:
        _NC_CACHE = build_nc()
    return _NC_CACHE


def kernel(output, target):
    nc = _get_nc()
    maps = host_shards(output, target)
    res = run_bass_kernel_spmd(nc, maps, core_ids=list(range(NCORES)))
    parts = [res.results[i]["partials"] for i in range(NCORES)]
    return combine(parts)


# revision 15
# speedup vs baseline: 1.0773x; 1.0773x over previous
"""RegionLoss (YOLO-style pose loss) on 8 Trainium2 NeuronCores.

Strategy: pure data parallel over the batch (16 images/core). The loss is
extremely sparse: the x/y terms and all metrics only touch the <=5 GT cells
per image, so each core does
  * a dense pass over just the conf channel (sigmoid -> sum sq, count>0.5),
  * one indirect-DMA gather of the 19 needed channel values at each GT cell
    (host passes a channels-last copy so each cell's channels are contiguous),
  * tiny per-GT vector math + PE column-reductions to 7 partial scalars.
Host sums the 8x7 partials (the psum step) and assembles the outputs.

All transcendentals use the natural_log+exp ACT table set (sigmoid via
exp+reciprocal, sqrt via exp(0.5*ln)) so only one table load occurs, early
and off the critical path. The per-slot "valid" cumprod is a PE matmul with
a host-built triangular prefix mask.
"""
import numpy as np
import concourse.bass as bass
import concourse.tile as tile
from concourse import bacc, mybir
from concourse.bass_utils import run_bass_kernel_spmd
from concourse.tile_rust import add_dep_helper

K = 9
NH = NW = 76
HW = NH * NW              # 5776
NBC = 16                  # images per core
MAXGT = 5
NG = NBC * MAXGT          # 80
C = 2 * K + 1 + 1         # 20 channels
NCH = 2 * K + 1           # 19 gathered channels (class ch unused)
IMG_STRIDE = C * HW       # 115520 elements per image (either layout)
P = 128
FREE = NBC * HW // P      # 722
NCORES = 8
B = NBC * NCORES          # 128
AX = (640.0 / 76.0) ** 2
AY = (480.0 / 76.0) ** 2
CC9 = float(1.0 / (np.exp(2.0) - 1.0 + 1e-5) / 9.0)
E2CC9 = float(np.exp(2.0) * CC9)   # ce = exp(-d/40)*E2CC9 - CC9

F32 = mybir.dt.float32
I32 = mybir.dt.int32

Act = None  # set in build


def build_nc():
    nc = bacc.Bacc(None)
    AF = mybir.ActivationFunctionType
    OP = mybir.AluOpType
    tgtb = nc.dram_tensor("tgtb", [NG, 2 * K + 1], F32, kind="ExternalInput")
    confb = nc.dram_tensor("confb", [P, FREE], F32, kind="ExternalInput")
    tri = nc.dram_tensor("tri", [P, P], F32, kind="ExternalInput")
    xoutt = nc.dram_tensor("xoutt", [NBC, NH, NW, C], F32, kind="ExternalInput")
    partials = nc.dram_tensor("partials", [5, 2], F32, kind="ExternalOutput")

    with tile.TileContext(nc) as tc:
        with tc.tile_pool(name="sb", bufs=1) as sb, \
             tc.tile_pool(name="ps", bufs=1, space="PSUM") as ps:

            # preload the one table set (natural_log_exp_and_others, id 6)
            # containing every ACT func we use: exp, ln, square
            ldset = mybir.InstLoadActFuncSet(
                name=nc.get_next_instruction_name(), act_func_set_id=6,
                ins=[], outs=[])
            nc.scalar.add_instruction(ldset)

            # ---------------- critical chain: targets -> idx -> gather ----------------
            tgt_t = sb.tile([NG, 2 * K + 1], F32)
            nc.sync.dma_start(out=tgt_t[:], in_=tgtb[:])
            conf_t = sb.tile([P, FREE], F32)
            nc.sync.dma_start(out=conf_t[:], in_=confb[:])

            # fused floor of (x0*76, y0*76): cols 0 and K of tgt_t via stride-K AP
            xy = sb.tile([NG, 2], F32)
            nc.vector.tensor_scalar_mul(out=xy[:], in0=tgt_t[:, 0:K + 1:K],
                                        scalar1=float(NW))
            xyi = sb.tile([NG, 2], I32)
            xyf = sb.tile([NG, 2], F32)
            fx = sb.tile([NG, 2], F32)
            nc.vector.tensor_copy(out=xyi[:], in_=xy[:])
            nc.vector.tensor_copy(out=xyf[:], in_=xyi[:])
            nc.vector.tensor_tensor(out=fx[:], in0=xyf[:], in1=xy[:], op=OP.is_gt)
            nc.vector.tensor_tensor(out=xyf[:], in0=xyf[:], in1=fx[:], op=OP.subtract)
            cxf, cyf = xyf[:, 0:1], xyf[:, 1:2]

            # idx = (cyf*76 + cxf)*20 + bofs  (channels-last element offsets; f32 exact)
            basef = sb.tile([NG, 1], F32)
            nc.vector.scalar_tensor_tensor(out=basef[:], in0=cyf, scalar=float(NW),
                                           in1=cxf, op0=OP.mult, op1=OP.add)
            idxf = sb.tile([NG, 1], F32)
            nc.vector.scalar_tensor_tensor(out=idxf[:], in0=basef[:], scalar=float(C),
                                           in1=tgt_t[:, 2 * K:2 * K + 1],
                                           op0=OP.mult, op1=OP.add)
            idx = sb.tile([NG, 1], I32)
            idx_inst = nc.vector.tensor_copy(out=idx[:], in_=idxf[:])

            def after_idx(inst):
                add_dep_helper(inst.ins, idx_inst.ins,
                               reason="defer off-critical DVE work past idx")
                return inst

            # full gx, gy (off critical path; used for tx/ty)
            gx = sb.tile([NG, K], F32)
            gy = sb.tile([NG, K], F32)
            after_idx(nc.vector.tensor_scalar_mul(out=gx[:], in0=tgt_t[:, 0:K],
                                                  scalar1=float(NW)))
            after_idx(nc.vector.tensor_scalar_mul(out=gy[:], in0=tgt_t[:, K:2 * K],
                                                  scalar1=float(NH)))

            # indirect gather: g_t[g, :] = xoutt.flat[idx[g] : idx[g]+19]
            # (HW semantics: one index per partition, contiguous run per index)
            g_t = sb.tile([NG, NCH], F32)
            xflat = xoutt[:].rearrange("b h w c -> b (h w c)")
            nc.gpsimd.indirect_dma_start(
                out=g_t[:], out_offset=None, in_=xflat,
                in_offset=bass.IndirectOffsetOnAxis(ap=idx[:], axis=1))

            # ---------------- post-gather per-GT math ----------------
            # channels-last layout: col 2k = x_k, col 2k+1 = y_k, col 18 = conf
            # sigmoid(v) = 1/(1+exp(-v)) on cols 0,1 (x0,y0) and 18 (conf)
            e01 = sb.tile([NG, 2], F32)
            nc.scalar.activation(out=e01[:], in_=g_t[:, 0:2], func=AF.Exp, scale=-1.0)
            nc.vector.tensor_scalar_add(out=e01[:], in0=e01[:], scalar1=1.0)
            nc.vector.reciprocal_approx_fast(out=g_t[:, 0:2], in_=e01[:])
            esc = sb.tile([NG, 1], F32)
            nc.scalar.activation(out=esc[:], in_=g_t[:, 2 * K:2 * K + 1], func=AF.Exp,
                                 scale=-1.0)
            nc.vector.tensor_scalar_add(out=esc[:], in0=esc[:], scalar1=1.0)
            sc = sb.tile([NG, 1], F32)
            nc.vector.reciprocal_approx_fast(out=sc[:], in_=esc[:])

            gvals = sb.tile([P, 5], F32)
            nc.vector.memset(gvals[:], 0.0)
            tx = sb.tile([NG, K], F32)
            ty = sb.tile([NG, K], F32)
            after_idx(nc.vector.tensor_scalar(out=tx[:], in0=gx[:], scalar1=cxf[:, 0:1],
                                              scalar2=None, op0=OP.subtract))
            after_idx(nc.vector.tensor_scalar(out=ty[:], in0=gy[:], scalar1=cyf[:, 0:1],
                                              scalar2=None, op0=OP.subtract))
            dx = sb.tile([NG, K], F32)
            dy = sb.tile([NG, K], F32)
            nc.vector.tensor_tensor(out=dx[:], in0=g_t[:, 0:2 * K:2], in1=tx[:],
                                    op=OP.subtract)
            nc.vector.tensor_tensor(out=dy[:], in0=g_t[:, 1:2 * K + 1:2], in1=ty[:],
                                    op=OP.subtract)
            dx2 = sb.tile([NG, K], F32)
            dy2 = sb.tile([NG, K], F32)
            nc.vector.scalar_tensor_tensor(
                out=dx2[:], in0=dx[:], scalar=1.0, in1=dx[:],
                op0=OP.mult, op1=OP.mult, accum_out=gvals[0:NG, 0:1])
            nc.vector.scalar_tensor_tensor(
                out=dy2[:], in0=dy[:], scalar=1.0, in1=dy[:],
                op0=OP.mult, op1=OP.mult, accum_out=gvals[0:NG, 1:2])

            # corner confidence: d = sqrt(AX*dx2 + AY*dy2) = exp(0.5*ln(s2))
            dy2b = sb.tile([NG, K], F32)
            nc.vector.tensor_scalar_mul(out=dy2b[:], in0=dy2[:], scalar1=AY)
            s2 = sb.tile([NG, K], F32)
            s2_inst = nc.vector.scalar_tensor_tensor(out=s2[:], in0=dx2[:], scalar=AX,
                                                     in1=dy2b[:], op0=OP.mult, op1=OP.add)
            lns = sb.tile([NG, K], F32)
            nc.scalar.activation(out=lns[:], in_=s2[:], func=AF.Ln)
            dd = sb.tile([NG, K], F32)
            nc.scalar.activation(out=dd[:], in_=lns[:], func=AF.Exp, scale=0.5)
            er = sb.tile([NG, K], F32)
            er_inst = nc.scalar.activation(out=er[:], in_=dd[:], func=AF.Exp,
                                           scale=-1.0 / 40.0)
            # mask d < 80  <=>  s2 < 6400
            mm = sb.tile([NG, K], F32)
            nc.vector.tensor_scalar(out=mm[:], in0=s2[:], scalar1=6400.0, scalar2=None,
                                    op0=OP.is_lt)
            # c/9 = (exp(2-d/40)-1)*CC9 = er*E2CC9 - CC9
            ce = sb.tile([NG, K], F32)
            nc.vector.tensor_scalar(out=ce[:], in0=er[:], scalar1=E2CC9, scalar2=CC9,
                                    op0=OP.mult, op1=OP.subtract)
            junk_g = sb.tile([NG, K], F32)
            confgt = sb.tile([NG, 1], F32)
            nc.vector.scalar_tensor_tensor(
                out=junk_g[:], in0=ce[:], scalar=1.0, in1=mm[:],
                op0=OP.mult, op1=OP.mult, accum_out=confgt[:])
            nc.vector.tensor_scalar(out=gvals[0:NG, 4:5], in0=confgt[:], scalar1=0.7,
                                    scalar2=None, op0=OP.is_gt)

            # conf correction 1 - 2*sigma(conf_logit)
            nc.vector.tensor_scalar(out=gvals[0:NG, 2:3], in0=sc[:], scalar1=-2.0,
                                    scalar2=1.0, op0=OP.mult, op1=OP.add)
            nc.vector.memset(gvals[0:NG, 3:4], 1.0)

            # ---------------- valid weights via PE prefix-count ----------------
            tri_t = sb.tile([P, P], F32)
            nc.sync.dma_start(out=tri_t[:], in_=tri[:])
            iz = sb.tile([P, 1], F32)
            nc.vector.memset(iz[:], 0.0)
            after_idx(nc.vector.tensor_scalar(out=iz[0:NG, 0:1], in0=tgt_t[:, 0:1],
                                              scalar1=0.0, scalar2=None, op0=OP.is_equal))
            psum_v = ps.tile([P, 1], F32)
            nc.tensor.matmul(out=psum_v[:], lhsT=tri_t[:], rhs=iz[:], start=True, stop=True)
            # valid = (prefix-zero-count == 0) = relu(1 - cnt); runs on ACT
            # (which can read PSUM) so it never stalls the DVE pipeline
            valid_w = sb.tile([P, 1], F32)
            one_b = nc.const_aps.tensor(1.0, (P, 1))
            nc.scalar.activation(out=valid_w[:], in_=psum_v[:], func=AF.Relu,
                                 scale=-1.0, bias=one_b)

            # ---------------- dense conf branch ----------------
            # dense sigma^2 sum: sigma = 1/(1+exp(-z)) with the fast custom-DVE
            # reciprocal (~51 ulp); count(z>0) = (FREE*P + sum sign(z))/2 rides
            # an ACT Sign pass (host un-affines it). One ACT pass + cheap DVE.
            dvals = sb.tile([P, 2], F32)
            ez = sb.tile([P, FREE], F32)
            nc.scalar.activation(out=ez[:], in_=conf_t[:], func=AF.Exp, scale=-1.0)
            junk_s = sb.tile([P, FREE], F32)
            nc.scalar.activation(out=junk_s[:], in_=conf_t[:], func=AF.Sign,
                                 accum_out=dvals[:, 1:2])
            nc.vector.tensor_scalar_add(out=ez[:], in0=ez[:], scalar1=1.0)
            sig = sb.tile([P, FREE], F32)
            nc.vector.reciprocal_approx_fast(out=sig[:], in_=ez[:])
            junk_d = sb.tile([P, FREE], F32)
            sq_inst = nc.vector.scalar_tensor_tensor(
                out=junk_d[:], in0=sig[:], scalar=1.0, in1=sig[:],
                op0=OP.mult, op1=OP.mult, accum_out=dvals[:, 0:1])
            add_dep_helper(sq_inst.ins, s2_inst.ins,
                           reason="dense sigma^2 reduce yields DVE to critical chain")

            ones = sb.tile([P, 1], F32)
            nc.vector.memset(ones[:], 1.0)

            # ---------------- reductions + output ----------------
            psum_g = ps.tile([5, 1], F32)
            psum_d = ps.tile([2, 1], F32)
            nc.tensor.matmul(out=psum_g[:], lhsT=gvals[:], rhs=valid_w[:],
                             start=True, stop=True)
            nc.tensor.matmul(out=psum_d[:], lhsT=dvals[:], rhs=ones[:],
                             start=True, stop=True)
            res = sb.tile([5, 2], F32)
            nc.vector.memset(res[:], 0.0)
            nc.vector.tensor_copy(out=res[0:5, 0:1], in_=psum_g[:])
            nc.vector.tensor_copy(out=res[0:2, 1:2], in_=psum_d[:])
            nc.sync.dma_start(out=partials[:], in_=res[:])
    nc.compile()
    return nc


def host_shards(output, target):
    """Split full inputs into per-core input maps (layout only, no math)."""
    output = np.ascontiguousarray(np.asarray(output, dtype=np.float32))
    target = np.ascontiguousarray(np.asarray(target, dtype=np.float32))
    g = np.arange(NG)
    bofs = ((g // MAXGT) * IMG_STRIDE).astype(np.float32)
    gb, gt_ = g[:, None] // MAXGT, g[:, None] % MAXGT
    tri = ((gb == gb.T) & (gt_ <= gt_.T)).astype(np.float32)
    tri_full = np.zeros((P, P), np.float32)
    tri_full[:NG, :NG] = tri
    maps = []
    for i in range(NCORES):
        ob = output[i * NBC:(i + 1) * NBC]
        confb = np.ascontiguousarray(ob[:, 2 * K].reshape(P, FREE))
        xoutt = np.ascontiguousarray(ob.transpose(0, 2, 3, 1))
        tb = target[i * NBC:(i + 1) * NBC].reshape(NBC, MAXGT, 2 * K + 3)
        tgtb = np.concatenate(
            [tb[:, :, 1:2 * K + 1:2], tb[:, :, 2:2 * K + 2:2]], axis=2
        ).reshape(NG, 2 * K)
        tgtb = np.ascontiguousarray(np.concatenate([tgtb, bofs[:, None]], axis=1))
        maps.append({"tgtb": tgtb, "confb": confb, "tri": tri_full, "xoutt": xoutt})
    return maps


def combine(partials_list):
    ps_ = np.stack([np.asarray(q).reshape(5, 2) for q in partials_list])
    p = ps_.sum(axis=0, dtype=np.float64).astype(np.float32)
    loss_x, loss_y, corr, ngt_cnt, ncorr_cnt = [np.float32(v) for v in p[:, 0]]
    sqsum = np.float32(p[0, 1])
    # col1 row1 holds sum(sign(z)) per core, summed over cores here
    prop_cnt = np.float32((NCORES * P * FREE + p[1, 1]) / 2.0)
    loss_conf = np.float32(sqsum + corr)
    loss = np.float32(np.float32(loss_x + loss_y) + loss_conf)
    nB = np.float32(B)
    return (loss, np.float32(ngt_cnt / nB), np.float32(ncorr_cnt / nB),
            np.float32(prop_cnt / nB), loss_x, loss_y, loss_conf)


_NC_CACHE = None


def _get_nc():
    global _NC_CACHE
    if _NC_CACHE is None:
        _NC_CACHE = build_nc()
    return _NC_CACHE


def kernel(output, target):
    nc = _get_nc()
    maps = host_shards(output, target)
    res = run_bass_kernel_spmd(nc, maps, core_ids=list(range(NCORES)))
    parts = [res.results[i]["partials"] for i in range(NCORES)]
    return combine(parts)


# revision 16
# speedup vs baseline: 1.1072x; 1.0278x over previous
"""RegionLoss (YOLO-style pose loss) on 8 Trainium2 NeuronCores.

Strategy: pure data parallel over the batch (16 images/core). The loss is
extremely sparse: the x/y terms and all metrics only touch the <=5 GT cells
per image, so each core does
  * a dense pass over just the conf channel (sigmoid -> sum sq, count>0.5),
  * one indirect-DMA gather of the 19 needed channel values at each GT cell
    (host passes a channels-last copy so each cell's channels are contiguous),
  * tiny per-GT vector math + PE column-reductions to 7 partial scalars.
Host sums the 8x7 partials (the psum step) and assembles the outputs.

All transcendentals use the natural_log+exp ACT table set (sigmoid via
exp+reciprocal, sqrt via exp(0.5*ln)) so only one table load occurs, early
and off the critical path. The per-slot "valid" cumprod is a PE matmul with
a host-built triangular prefix mask.
"""
import numpy as np
import concourse.bass as bass
import concourse.tile as tile
from concourse import bacc, mybir
from concourse.bass_utils import run_bass_kernel_spmd
from concourse.tile_rust import add_dep_helper

K = 9
NH = NW = 76
HW = NH * NW              # 5776
NBC = 16                  # images per core
MAXGT = 5
NG = NBC * MAXGT          # 80
C = 2 * K + 1 + 1         # 20 channels
NCH = 2 * K + 1           # 19 gathered channels (class ch unused)
IMG_STRIDE = C * HW       # 115520 elements per image (either layout)
P = 128
FREE = NBC * HW // P      # 722
NCORES = 8
B = NBC * NCORES          # 128
AX = (640.0 / 76.0) ** 2
AY = (480.0 / 76.0) ** 2
CC9 = float(1.0 / (np.exp(2.0) - 1.0 + 1e-5) / 9.0)
E2CC9 = float(np.exp(2.0) * CC9)   # ce = exp(-d/40)*E2CC9 - CC9

F32 = mybir.dt.float32
I32 = mybir.dt.int32

Act = None  # set in build


def build_nc():
    nc = bacc.Bacc(None)
    AF = mybir.ActivationFunctionType
    OP = mybir.AluOpType
    tgtb = nc.dram_tensor("tgtb", [NG, 2 * K + 1], F32, kind="ExternalInput")
    confb = nc.dram_tensor("confb", [P, FREE], F32, kind="ExternalInput")
    tri = nc.dram_tensor("tri", [P, P], F32, kind="ExternalInput")
    xoutt = nc.dram_tensor("xoutt", [NBC, NH, NW, C], F32, kind="ExternalInput")
    partials = nc.dram_tensor("partials", [5, 2], F32, kind="ExternalOutput")

    with tile.TileContext(nc) as tc:
        with tc.tile_pool(name="sb", bufs=1) as sb, \
             tc.tile_pool(name="ps", bufs=1, space="PSUM") as ps:

            # preload the one table set (natural_log_exp_and_others, id 6)
            # containing every ACT func we use: exp, ln, square
            ldset = mybir.InstLoadActFuncSet(
                name=nc.get_next_instruction_name(), act_func_set_id=6,
                ins=[], outs=[])
            nc.scalar.add_instruction(ldset)

            # ---------------- critical chain: targets -> idx -> gather ----------------
            tgt_t = sb.tile([NG, 2 * K + 1], F32)
            nc.sync.dma_start(out=tgt_t[:], in_=tgtb[:])
            conf_t = sb.tile([P, FREE], F32)
            nc.sync.dma_start(out=conf_t[:], in_=confb[:])

            # fused floor of (x0*76, y0*76): cols 0 and K of tgt_t via stride-K AP
            xy = sb.tile([NG, 2], F32)
            nc.vector.tensor_scalar_mul(out=xy[:], in0=tgt_t[:, 0:K + 1:K],
                                        scalar1=float(NW))
            xyi = sb.tile([NG, 2], I32)
            xyf = sb.tile([NG, 2], F32)
            fx = sb.tile([NG, 2], F32)
            nc.vector.tensor_copy(out=xyi[:], in_=xy[:])
            nc.vector.tensor_copy(out=xyf[:], in_=xyi[:])
            nc.vector.tensor_tensor(out=fx[:], in0=xyf[:], in1=xy[:], op=OP.is_gt)
            nc.vector.tensor_tensor(out=xyf[:], in0=xyf[:], in1=fx[:], op=OP.subtract)
            cxf, cyf = xyf[:, 0:1], xyf[:, 1:2]

            # idx = (cyf*76 + cxf)*20 + bofs  (channels-last element offsets; f32 exact)
            basef = sb.tile([NG, 1], F32)
            nc.vector.scalar_tensor_tensor(out=basef[:], in0=cyf, scalar=float(NW),
                                           in1=cxf, op0=OP.mult, op1=OP.add)
            idxf = sb.tile([NG, 1], F32)
            nc.vector.scalar_tensor_tensor(out=idxf[:], in0=basef[:], scalar=float(C),
                                           in1=tgt_t[:, 2 * K:2 * K + 1],
                                           op0=OP.mult, op1=OP.add)
            idx = sb.tile([NG, 1], I32)
            idx_inst = nc.vector.tensor_copy(out=idx[:], in_=idxf[:])

            def after_idx(inst):
                add_dep_helper(inst.ins, idx_inst.ins,
                               reason="defer off-critical DVE work past idx")
                return inst

            # full gx, gy (off critical path; used for tx/ty)
            gx = sb.tile([NG, K], F32)
            gy = sb.tile([NG, K], F32)
            after_idx(nc.vector.tensor_scalar_mul(out=gx[:], in0=tgt_t[:, 0:K],
                                                  scalar1=float(NW)))
            after_idx(nc.vector.tensor_scalar_mul(out=gy[:], in0=tgt_t[:, K:2 * K],
                                                  scalar1=float(NH)))

            # indirect gather: g_t[g, :] = xoutt.flat[idx[g] : idx[g]+20]
            # (HW semantics: one index per partition, contiguous run per index)
            g_t = sb.tile([NG, C], F32)
            xflat = xoutt[:].rearrange("b h w c -> b (h w c)")
            nc.gpsimd.indirect_dma_start(
                out=g_t[:], out_offset=None, in_=xflat,
                in_offset=bass.IndirectOffsetOnAxis(ap=idx[:], axis=1))

            # ---------------- post-gather per-GT math ----------------
            # channels-last layout: col 2k = x_k, col 2k+1 = y_k, col 18 = conf.
            # One fused sigmoid over cols {0,1,18,19} (x0, y0, conf, junk class)
            # via a [80,2,2] access pattern.
            quad = g_t[:].rearrange("g (a c) -> g a c", c=2)[:, 0:10:9, :]
            e4 = sb.tile([NG, 4], F32)
            e43 = e4[:].rearrange("g (a c) -> g a c", c=2)
            nc.scalar.activation(out=e43, in_=quad, func=AF.Exp, scale=-1.0)
            nc.vector.tensor_scalar_add(out=e4[:], in0=e4[:], scalar1=1.0)
            sig4 = sb.tile([NG, 4], F32)
            nc.vector.reciprocal_approx_fast(out=sig4[:], in_=e4[:])
            nc.vector.tensor_copy(out=g_t[:, 0:2], in_=sig4[:, 0:2])
            sc = sig4[:, 2:3]

            gvals = sb.tile([P, 5], F32)
            nc.vector.memset(gvals[:], 0.0)
            tx = sb.tile([NG, K], F32)
            ty = sb.tile([NG, K], F32)
            after_idx(nc.vector.tensor_scalar(out=tx[:], in0=gx[:], scalar1=cxf[:, 0:1],
                                              scalar2=None, op0=OP.subtract))
            after_idx(nc.vector.tensor_scalar(out=ty[:], in0=gy[:], scalar1=cyf[:, 0:1],
                                              scalar2=None, op0=OP.subtract))
            dx = sb.tile([NG, K], F32)
            dy = sb.tile([NG, K], F32)
            nc.vector.tensor_tensor(out=dx[:], in0=g_t[:, 0:2 * K:2], in1=tx[:],
                                    op=OP.subtract)
            nc.vector.tensor_tensor(out=dy[:], in0=g_t[:, 1:2 * K + 1:2], in1=ty[:],
                                    op=OP.subtract)
            dx2 = sb.tile([NG, K], F32)
            dy2 = sb.tile([NG, K], F32)
            nc.vector.scalar_tensor_tensor(
                out=dx2[:], in0=dx[:], scalar=1.0, in1=dx[:],
                op0=OP.mult, op1=OP.mult, accum_out=gvals[0:NG, 0:1])
            nc.vector.scalar_tensor_tensor(
                out=dy2[:], in0=dy[:], scalar=1.0, in1=dy[:],
                op0=OP.mult, op1=OP.mult, accum_out=gvals[0:NG, 1:2])

            # corner confidence: d = sqrt(AX*dx2 + AY*dy2) = exp(0.5*ln(s2))
            dy2b = sb.tile([NG, K], F32)
            nc.vector.tensor_scalar_mul(out=dy2b[:], in0=dy2[:], scalar1=AY)
            s2 = sb.tile([NG, K], F32)
            s2_inst = nc.vector.scalar_tensor_tensor(out=s2[:], in0=dx2[:], scalar=AX,
                                                     in1=dy2b[:], op0=OP.mult, op1=OP.add)
            lns = sb.tile([NG, K], F32)
            nc.scalar.activation(out=lns[:], in_=s2[:], func=AF.Ln)
            dd = sb.tile([NG, K], F32)
            nc.scalar.activation(out=dd[:], in_=lns[:], func=AF.Exp, scale=0.5)
            er = sb.tile([NG, K], F32)
            er_inst = nc.scalar.activation(out=er[:], in_=dd[:], func=AF.Exp,
                                           scale=-1.0 / 40.0)
            # c/9 masked by (d<80) = relu(er*E2CC9 - CC9): the mask condition
            # d<80 is exactly ce>0. One ACT op with free-axis accumulate.
            negcc = sb.tile([NG, 1], F32)
            nc.vector.memset(negcc[:], -CC9)
            junk_g = sb.tile([NG, K], F32)
            confgt = sb.tile([NG, 1], F32)
            nc.scalar.activation(out=junk_g[:], in_=er[:], func=AF.Relu,
                                 scale=E2CC9, bias=negcc[:], accum_out=confgt[:])
            nc.vector.tensor_scalar(out=gvals[0:NG, 4:5], in0=confgt[:], scalar1=0.7,
                                    scalar2=None, op0=OP.is_gt)

            # conf correction 1 - 2*sigma(conf_logit)
            nc.vector.tensor_scalar(out=gvals[0:NG, 2:3], in0=sc[:], scalar1=-2.0,
                                    scalar2=1.0, op0=OP.mult, op1=OP.add)
            nc.vector.memset(gvals[0:NG, 3:4], 1.0)

            # ---------------- valid weights via PE prefix-count ----------------
            tri_t = sb.tile([P, P], F32)
            nc.sync.dma_start(out=tri_t[:], in_=tri[:])
            iz = sb.tile([P, 1], F32)
            nc.vector.memset(iz[:], 0.0)
            after_idx(nc.vector.tensor_scalar(out=iz[0:NG, 0:1], in0=tgt_t[:, 0:1],
                                              scalar1=0.0, scalar2=None, op0=OP.is_equal))
            psum_v = ps.tile([P, 1], F32)
            nc.tensor.matmul(out=psum_v[:], lhsT=tri_t[:], rhs=iz[:], start=True, stop=True)
            # valid = (prefix-zero-count == 0) = relu(1 - cnt); runs on ACT
            # (which can read PSUM) so it never stalls the DVE pipeline
            valid_w = sb.tile([P, 1], F32)
            one_b = nc.const_aps.tensor(1.0, (P, 1))
            nc.scalar.activation(out=valid_w[:], in_=psum_v[:], func=AF.Relu,
                                 scale=-1.0, bias=one_b)

            # ---------------- dense conf branch ----------------
            # dense sigma^2 sum: sigma = 1/(1+exp(-z)) with the fast custom-DVE
            # reciprocal (~51 ulp); count(z>0) = (FREE*P + sum sign(z))/2 rides
            # an ACT Sign pass (host un-affines it). One ACT pass + cheap DVE.
            dvals = sb.tile([P, 2], F32)
            ez = sb.tile([P, FREE], F32)
            nc.scalar.activation(out=ez[:], in_=conf_t[:], func=AF.Exp, scale=-1.0)
            junk_s = sb.tile([P, FREE], F32)
            nc.scalar.activation(out=junk_s[:], in_=conf_t[:], func=AF.Sign,
                                 accum_out=dvals[:, 1:2])
            nc.vector.tensor_scalar_add(out=ez[:], in0=ez[:], scalar1=1.0)
            sig = sb.tile([P, FREE], F32)
            nc.vector.reciprocal_approx_fast(out=sig[:], in_=ez[:])
            junk_d = sb.tile([P, FREE], F32)
            sq_inst = nc.vector.scalar_tensor_tensor(
                out=junk_d[:], in0=sig[:], scalar=1.0, in1=sig[:],
                op0=OP.mult, op1=OP.mult, accum_out=dvals[:, 0:1])
            add_dep_helper(sq_inst.ins, s2_inst.ins,
                           reason="dense sigma^2 reduce yields DVE to critical chain")

            ones = sb.tile([P, 1], F32)
            nc.vector.memset(ones[:], 1.0)

            # ---------------- reductions + output ----------------
            psum_g = ps.tile([5, 1], F32)
            psum_d = ps.tile([2, 1], F32)
            nc.tensor.matmul(out=psum_g[:], lhsT=gvals[:], rhs=valid_w[:],
                             start=True, stop=True)
            nc.tensor.matmul(out=psum_d[:], lhsT=dvals[:], rhs=ones[:],
                             start=True, stop=True)
            res = sb.tile([5, 2], F32)
            nc.vector.memset(res[:], 0.0)
            nc.vector.tensor_copy(out=res[0:5, 0:1], in_=psum_g[:])
            nc.vector.tensor_copy(out=res[0:2, 1:2], in_=psum_d[:])
            nc.sync.dma_start(out=partials[:], in_=res[:])
    nc.compile()
    return nc


def host_shards(output, target):
    """Split full inputs into per-core input maps (layout only, no math)."""
    output = np.ascontiguousarray(np.asarray(output, dtype=np.float32))
    target = np.ascontiguousarray(np.asarray(target, dtype=np.float32))
    g = np.arange(NG)
    bofs = ((g // MAXGT) * IMG_STRIDE).astype(np.float32)
    gb, gt_ = g[:, None] // MAXGT, g[:, None] % MAXGT
    tri = ((gb == gb.T) & (gt_ <= gt_.T)).astype(np.float32)
    tri_full = np.zeros((P, P), np.float32)
    tri_full[:NG, :NG] = tri
    maps = []
    for i in range(NCORES):
        ob = output[i * NBC:(i + 1) * NBC]
        confb = np.ascontiguousarray(ob[:, 2 * K].reshape(P, FREE))
        xoutt = np.ascontiguousarray(ob.transpose(0, 2, 3, 1))
        tb = target[i * NBC:(i + 1) * NBC].reshape(NBC, MAXGT, 2 * K + 3)
        tgtb = np.concatenate(
            [tb[:, :, 1:2 * K + 1:2], tb[:, :, 2:2 * K + 2:2]], axis=2
        ).reshape(NG, 2 * K)
        tgtb = np.ascontiguousarray(np.concatenate([tgtb, bofs[:, None]], axis=1))
        maps.append({"tgtb": tgtb, "confb": confb, "tri": tri_full, "xoutt": xoutt})
    return maps


def combine(partials_list):
    ps_ = np.stack([np.asarray(q).reshape(5, 2) for q in partials_list])
    p = ps_.sum(axis=0, dtype=np.float64).astype(np.float32)
    loss_x, loss_y, corr, ngt_cnt, ncorr_cnt = [np.float32(v) for v in p[:, 0]]
    sqsum = np.float32(p[0, 1])
    # col1 row1 holds sum(sign(z)) per core, summed over cores here
    prop_cnt = np.float32((NCORES * P * FREE + p[1, 1]) / 2.0)
    loss_conf = np.float32(sqsum + corr)
    loss = np.float32(np.float32(loss_x + loss_y) + loss_conf)
    nB = np.float32(B)
    return (loss, np.float32(ngt_cnt / nB), np.float32(ncorr_cnt / nB),
            np.float32(prop_cnt / nB), loss_x, loss_y, loss_conf)


_NC_CACHE = None


def _get_nc():
    global _NC_CACHE
    if _NC_CACHE is None:
        _NC_CACHE = build_nc()
    return _NC_CACHE


def kernel(output, target):
    nc = _get_nc()
    maps = host_shards(output, target)
    res = run_bass_kernel_spmd(nc, maps, core_ids=list(range(NCORES)))
    parts = [res.results[i]["partials"] for i in range(NCORES)]
    return combine(parts)
